# revision 14
# baseline (speedup 1.0000x reference)
"""Causal attention (QKV proj + softmax(QK^T/sqrt(d))V) on 8 TRN2 NeuronCores.

Sharding: data-parallel over batch (B=8, one batch element per core).
Per-core kernel: matmul operands bf16 (fp32 PSUM accumulation), x enters
through f32r PE transposes whose PSUM evictions downconvert to bf16 (the
fast cast path; SBUF->SBUF casts are slow, so weight downconverts run on
gpsimd far ahead of their deadlines, after gpsimd's phase-0 DMA triggers):
  phase 0: x [T,D] f32r -> x^T stored t-block-major [P, tb, dc, 128] bf16
           via PE transposes, 4 transposes batched per PSUM bank so one
           [P,512] eviction covers them.
  phase 1: Q^T and K^T both SBUF resident in bf16 (no DRAM roundtrip);
           then V = x @ Wv evicted IN PLACE over the x^T slab of the same
           t-block (x^T morphs into resident V). Projection loops are
           dc-outer/tsl-inner so consecutive matmuls reuse the stationary
           operand (skips LDWEIGHTS).
  phase 2: per 512-wide query supertile: S^T = K Q^T (probs produced
           directly in the lhsT layout needed by P@V) with diagonal
           blocks trimmed to causal width, exp on ACT with fused
           1/sqrt(D) scale, 128-wide triangle mask on the diagonal block,
           then per query block one interleaved (PV_e0, PV_e1, rowsum)
           matmul triple sharing each stationary P^T block, reciprocal
           normalize split over ACT/DVE, stores rotating over 3 DMA rings.
"""

import numpy as np

T = 2048
D = 1024
E = 1024
N_CORES = 8
P = 128
TS = 512  # t-slice / supertile width
SCALE = 1.0 / 32.0  # 1/sqrt(D)

DC = D // P  # 8 d-chunks
EC = E // P  # 8 e-chunks
TB = T // P  # 16 t-blocks of 128
NTS = T // TS  # 4 t-slices of 512
JB = TS // P  # 4 q-blocks per supertile
QB = TB // 4  # pT part size in k-blocks


def _attention_kernel(ctx, tc, out, x, wq, wk, wv):
    import concourse.bass as bass
    from concourse import mybir
    from concourse.bass import ts
    from concourse.masks import make_identity

    nc = tc.nc
    f32 = mybir.dt.float32
    f32r = mybir.dt.float32r
    bf16 = mybir.dt.bfloat16
    AF = mybir.ActivationFunctionType

    # ---- left-side SBUF pools ----
    const = ctx.enter_context(tc.tile_pool(name="const", bufs=1))
    # identity first: the first transpose needs it ~7us in
    identity_f32 = const.tile([P, P], f32)
    make_identity(nc, identity_f32[:])
    identity = const.tile([P, P], f32r)
    nc.vector.tensor_copy(identity[:], identity_f32[:])
    ones_f32 = const.tile([P, 2], f32)
    nc.vector.memset(ones_f32[:], 1.0)
    ones_col = const.tile([P, 2], bf16)
    nc.vector.tensor_copy(ones_col[:], ones_f32[:])
    # warm the ACT exp table set at program start (off the critical path)
    exp_warm = const.tile([P, 2], f32)
    nc.scalar.activation(exp_warm[:], ones_f32[:], AF.Exp)
    tri_f32 = const.tile([P, P], f32)
    tri = const.tile([P, P], bf16)

    kt_pool = ctx.enter_context(tc.tile_pool(name="ktres", bufs=1))
    KT = kt_pool.tile([P, EC, T], bf16)  # K^T[e, t], e = ec*128 + ep
    qt_pool = ctx.enter_context(tc.tile_pool(name="qtres", bufs=1))
    QT = qt_pool.tile([P, EC, T], bf16)  # Q^T[e, t], resident

    # ---- right-side work pools ----
    tc.swap_default_side()
    xv_pool = ctx.enter_context(tc.tile_pool(name="xv", bufs=1))
    # x^T t-block-major; after phase 1 each slab is overwritten in place
    # with V[tb] so this same tile is the resident V in phase 2.
    xv = xv_pool.tile([P, TB, DC, P], bf16)  # [dp, tb, dc, tl]
    Vres = xv[:].rearrange("p tb dc e -> p tb (dc e)")  # V[t, e] view
    wvb_pool = tc.alloc_tile_pool(name="wvb", bufs=1)
    wvb = wvb_pool.tile([P, DC, E], bf16)  # Wv resident bf16 [dp, dc, e]
    xstg = tc.alloc_tile_pool(name="xstg", bufs=3)
    wstg_pool = tc.alloc_tile_pool(name="wstg", bufs=7)
    wb_pool = tc.alloc_tile_pool(name="wb", bufs=9)
    tc.swap_default_side()

    # ---- PSUM pools for phases 0/1 ----
    ps_tp = tc.alloc_tile_pool(name="ps_tp", bufs=2, space="PSUM")
    ps_proj = tc.alloc_tile_pool(name="ps_proj", bufs=6, space="PSUM")

    wq_view = wq.rearrange("(dc dp) e -> dp dc e", dp=P)
    wk_view = wk.rearrange("(dc dp) e -> dp dc e", dp=P)
    wv_view = wv.rearrange("(dc dp) e -> dp dc e", dp=P)

    wst_tiles = {}

    def w_trig(view, name, ring):
        wst = wstg_pool.tile([P, DC, P], f32, tag="wstg", name=f"wst_{name}")
        ring.dma_start(wst[:], view[:, :, ts(int(name[1]), P)])
        wst_tiles[name] = wst

    def w_cast(name, dst=None):
        wst = wst_tiles.pop(name)
        if dst is None:
            dst = wb_pool.tile([P, DC, P], bf16, tag="wb", name=f"wb_{name}")
            nc.gpsimd.tensor_copy(dst[:], wst[:])
            return dst
        nc.gpsimd.tensor_copy(dst, wst[:])
        return None

    # ===== phase 0: x -> x^T via f32r PE transposes, evict to bf16 =====
    for tb in range(TB):
        xa = xstg.tile([P, D], f32r, tag="xstg", name=f"xa_{tb}")
        # gpsimd only triggers the two early tiles sync/scalar can't cover;
        # afterwards its FIFO is free to run the weight casts from ~8us on
        if tb in (1, 4):
            eng = nc.gpsimd
        else:
            eng = (nc.sync, nc.scalar)[tb % 2]
        if tb < 2:
            # per-dc loads so the first transposes start ~4x earlier
            for dc in range(DC):
                eng.dma_start(
                    xa[:, ts(dc, P)], x[ts(tb, P), ts(dc, P)].bitcast(f32r)
                )
        else:
            eng.dma_start(
                xa[:, 0 : D // 2], x[ts(tb, P), 0 : D // 2].bitcast(f32r)
            )
            eng.dma_start(
                xa[:, D // 2 : D], x[ts(tb, P), D // 2 : D].bitcast(f32r)
            )
        # stage Wq chunks 0..3 through phase 0; scalar's ring is free
        # until its first x tile (tb2), sync's until tb3
        if tb == 0:
            w_trig(wq_view, "q0", nc.scalar)
            w_trig(wq_view, "q1", nc.scalar)
        elif tb == 5:
            w_trig(wq_view, "q2", nc.sync)
        elif tb == 9:
            w_trig(wq_view, "q3", nc.scalar)
        for dh in range(2):
            ps4 = ps_tp.tile([P, 4, P], f32r, tag="ptp", name=f"ptp_{tb}_{dh}")
            for i in range(4):
                nc.tensor.transpose(
                    ps4[:, i, :], xa[:, ts(4 * dh + i, P)], identity[:]
                )
            if dh == 0:
                nc.vector.tensor_copy(xv[:, tb, 0:4, :], ps4[:])
            else:
                nc.scalar.copy(xv[:, tb, 4:8, :], ps4[:])

    # gpsimd's x triggers are done; queue its weight downconverts now.
    # Each self-paces on its staging DMA semaphore. Deadline of wqb[i] is
    # the start of Q projection group i (~45 + 7.2*i us); casts run ~3.6us.
    wqb, wkb = {}, {}
    wqb[0] = w_cast("q0")

    # ======== phase 1a: Q^T and K^T, both SBUF resident ========
    # dc-outer / tsl-inner: the stationary W chunk is loaded once per dc
    # and reused across the 4 interleaved psum accumulation groups.
    for w_name, dest in (("q", QT), ("k", KT)):
        for eb in range(EC):
            if w_name == "q":
                # stage Wq 4 groups ahead, Wk per-group, Wv 4 ahead of its
                # K-loop slot; cast Wq 1 group ahead. Wk/Wv casts are
                # emitted only after all Wq casts so gpsimd's FIFO serves
                # the tightest deadlines first.
                nb = eb + 4
                if nb < EC:
                    w_trig(wq_view, f"q{nb}", (nc.sync, nc.scalar)[nb % 2])
                w_trig(wk_view, f"k{eb}", (nc.scalar, nc.sync)[eb % 2])
                if eb >= 4:
                    w_trig(wv_view, f"v{eb - 4}", (nc.sync, nc.scalar)[eb % 2])
                if eb + 1 < EC:
                    wqb[eb + 1] = w_cast(f"q{eb + 1}")
                wrb = wqb.pop(eb)
            else:
                if eb == 0:
                    for kb in range(EC):  # all Wk casts, after the Wq ones
                        wkb[kb] = w_cast(f"k{kb}")
                if eb < 4:
                    w_trig(wv_view, f"v{eb + 4}", (nc.sync, nc.scalar)[eb % 2])
                if eb == EC - 1:
                    for vb in range(EC):  # all Wv casts, last deadlines
                        w_cast(f"v{vb}", dst=wvb[:, :, ts(vb, P)])
                wrb = wkb.pop(eb)
            pps = [
                ps_proj.tile([P, TS], f32, tag="pp", name=f"pp_{w_name}{eb}_{t}")
                for t in range(NTS)
            ]
            for dc in range(DC):
                for tsl in range(NTS):
                    nc.tensor.matmul(
                        pps[tsl][:],
                        wrb[:, dc, :],
                        xv[:, 4 * tsl : 4 * tsl + 4, dc, :],
                        start=(dc == 0),
                        stop=(dc == DC - 1),
                    )
            for tsl in range(NTS):
                if tsl % 2 == 0:
                    nc.vector.tensor_copy(dest[:, eb, ts(tsl, TS)], pps[tsl][:])
                else:
                    nc.scalar.copy(dest[:, eb, ts(tsl, TS)], pps[tsl][:])

    # causal triangle for the 128-wide diagonal band: keep where q - p >= 0
    # (queued on gpsimd after the weight casts; needed only in phase 2)
    nc.gpsimd.memset(tri_f32[:], 1.0)
    nc.gpsimd.affine_select(
        out=tri_f32[:],
        in_=tri_f32[:],
        compare_op=mybir.AluOpType.is_ge,
        fill=0.0,
        base=0,
        pattern=[[1, P]],
        channel_multiplier=-1,
    )
    nc.vector.tensor_copy(tri[:], tri_f32[:])

    # ========== phase 1b: V = x @ Wv, evicted in place over x^T ==========
    # tb-outer; both psums must be computed before the in-place evicts may
    # overwrite this t-block's x^T slab. eh-inner so the stationary x^T
    # block is loaded once per dc.
    for tb in range(TB):
        pps = [
            ps_proj.tile([P, TS], f32, tag="pp", name=f"ppv_{tb}_{t}")
            for t in range(2)
        ]
        for dc in range(DC):
            for eh in range(2):
                nc.tensor.matmul(
                    pps[eh][:],
                    xv[:, tb, dc, :],
                    wvb[:, dc, ts(eh, TS)],
                    start=(dc == 0),
                    stop=(dc == DC - 1),
                )
        # in-place evicts over the x^T slab of this t-block (WAR: both
        # psum groups above have read the slab before these run)
        nc.scalar.copy(Vres[:, tb, ts(0, TS)], pps[0][:])
        nc.vector.tensor_copy(Vres[:, tb, ts(1, TS)], pps[1][:])

    wb_pool.release()
    wstg_pool.release()
    xstg.release()
    wvb_pool.release()
    ps_proj.release()
    ps_tp.release()

    # ================= phase 2: attention =================
    ps_s = tc.alloc_tile_pool(name="ps_s", bufs=2, space="PSUM")
    ps_o = tc.alloc_tile_pool(name="ps_o", bufs=4, space="PSUM")
    ps_sum = tc.alloc_tile_pool(name="ps_sum", bufs=2, space="PSUM")

    tc.swap_default_side()
    pt_pool = ctx.enter_context(tc.tile_pool(name="pt", bufs=8))
    rs_pool = ctx.enter_context(tc.tile_pool(name="rs", bufs=4))
    ostg = ctx.enter_context(tc.tile_pool(name="ostg", bufs=4))
    tc.swap_default_side()

    for sup in range(NTS):
        nkb = JB * sup + JB  # key blocks 0..nkb-1
        pt_parts = [
            pt_pool.tile([P, QB, TS], bf16, tag="pt", name=f"ptp_{sup}_0")
        ]

        # --- S^T blocks + exp + causal triangle on the diagonal band ---
        # diagonal block j computes only query columns [128j, 512): columns
        # below that are never read by P@V (query block jq >= j only).
        for k in range(nkb):
            j = k - JB * sup
            j0 = max(j, 0) * P
            ssp = ps_s.tile([P, TS], f32, tag="ssp", name=f"ssp_{sup}_{k}")
            for ec in range(EC):
                nc.tensor.matmul(
                    ssp[:, j0:TS],
                    KT[:, ec, ts(k, P)],
                    QT[:, ec, sup * TS + j0 : (sup + 1) * TS],
                    start=(ec == 0),
                    stop=(ec == EC - 1),
                )
            if k // QB >= len(pt_parts):
                pt_parts.append(
                    pt_pool.tile(
                        [P, QB, TS], bf16, tag="pt",
                        name=f"ptp_{sup}_{k // QB}",
                    )
                )
            pk = pt_parts[k // QB][:, k % QB, :]
            nc.scalar.activation(pk[:, j0:TS], ssp[:, j0:TS], AF.Exp, scale=SCALE)
            if j >= 0:
                nc.vector.tensor_mul(pk[:, j0 : j0 + P], pk[:, j0 : j0 + P], tri[:])

        # --- P @ V: per query block, (PV_e0, PV_e1, rowsum) triples share
        # each stationary P^T block; reciprocal normalize, store ---
        for jq in range(JB):
            qb = JB * sup + jq
            nk = qb + 1
            po = [
                ps_o.tile([P, TS], f32, tag="po", name=f"po_{qb}_{t}")
                for t in range(2)
            ]
            pos = ps_sum.tile([P, 2], f32, tag="pos", name=f"pos_{qb}")
            for k in range(nk):
                lhsT = pt_parts[k // QB][:, k % QB, ts(jq, P)]
                for eh in range(2):
                    nc.tensor.matmul(
                        po[eh][:],
                        lhsT,
                        Vres[:, k, ts(eh, TS)],
                        start=(k == 0),
                        stop=(k == nk - 1),
                    )
                nc.tensor.matmul(
                    pos[:],
                    lhsT,
                    ones_col[:],
                    start=(k == 0),
                    stop=(k == nk - 1),
                )
            rs = rs_pool.tile([P, 1], f32, tag="rs")
            nc.vector.reciprocal(rs[:], pos[:, 0:1])
            for eh in range(2):
                ost = ostg.tile([P, TS], f32, tag="ostage")
                if eh == 0:
                    nc.scalar.activation(ost[:], po[eh][:], AF.Copy, scale=rs[:])
                else:
                    nc.vector.tensor_scalar_mul(ost[:], po[eh][:], rs[:])
                ring = (nc.scalar, nc.gpsimd, nc.sync)[(2 * qb + eh) % 3]
                ring.dma_start(out[ts(qb, P), ts(eh, TS)], ost[:])

    ps_sum.release()
    ps_o.release()
    ps_s.release()


def build_program():
    from contextlib import ExitStack

    import concourse.bacc as bacc
    import concourse.tile as tile
    from concourse import mybir

    nc = bacc.Bacc("TRN2", target_bir_lowering=False, debug=False)
    f32 = mybir.dt.float32
    x = nc.dram_tensor("x", [T, D], f32, kind="ExternalInput").ap()
    wq = nc.dram_tensor("Wq", [D, E], f32, kind="ExternalInput").ap()
    wk = nc.dram_tensor("Wk", [D, E], f32, kind="ExternalInput").ap()
    wv = nc.dram_tensor("Wv", [D, E], f32, kind="ExternalInput").ap()
    out = nc.dram_tensor("out", [T, E], f32, kind="ExternalOutput").ap()

    with tile.TileContext(nc) as tc:
        with ExitStack() as ctx:
            _attention_kernel(ctx, tc, out, x, wq, wk, wv)
    nc.compile()
    return nc


def kernel(x, Wq, Wk, Wv, _trace=False):
    from concourse.bass_utils import run_bass_kernel_spmd

    x = np.ascontiguousarray(np.asarray(x), dtype=np.float32)
    Wq = np.ascontiguousarray(np.asarray(Wq), dtype=np.float32)
    Wk = np.ascontiguousarray(np.asarray(Wk), dtype=np.float32)
    Wv = np.ascontiguousarray(np.asarray(Wv), dtype=np.float32)
    assert x.shape == (N_CORES, T, D), x.shape

    nc = build_program()
    in_maps = [
        {"x": np.ascontiguousarray(x[b]), "Wq": Wq, "Wk": Wk, "Wv": Wv}
        for b in range(N_CORES)
    ]
    last_err = None
    for attempt in range(3):
        try:
            res = run_bass_kernel_spmd(
                nc, in_maps, core_ids=list(range(N_CORES)), trace=_trace
            )
            break
        except Exception as e:  # transient device wedge: retry
            last_err = e
            import time

            time.sleep(5.0 * (attempt + 1))
    else:
        raise last_err
    out = np.stack([res.results[b]["out"] for b in range(N_CORES)], axis=0)
    if _trace:
        kernel.last_results = res
    return out


kernel.last_results = None
